# revision 16
# baseline (speedup 1.0000x reference)
"""Bidirectional LSTM encoder (nn_BiEncode) as a Bass/Tile kernel on 8 trn2 cores.

Sharding: direction-split x batch-split. Cores 0-3 run the LEFT (forward-time)
direction on batch shards 0-3 (512 rows each); cores 4-7 run the RIGHT
direction (time-reversed input, handled host-side) on the same batch shards.
Every core runs the identical SPMD program; direction differences live
entirely in the data it is fed (weights + time-reversed x).

Device layout: everything is kept "transposed" (feature dim on partitions,
batch on the free dim) so the scan needs no on-chip transposes:
  x fed as xT[t, i, b], weights as W^T, h/c as [H, B] tiles, output written
  as yT[t, h, b] and un-transposed on the host.

Per timestep the full gate pre-activation g^T[4H, B] is computed as 12
PSUM-accumulated matmuls per 128-row gate tile (8 k-tiles of x-projection +
4 k-tiles of the recurrent term) -- the input projection is fused into the
scan, so no pre-activation tensor is ever materialized. ACT applies
sigmoid/tanh straight out of PSUM; DVE does the cell update.
"""

import os

import numpy as np

FRAME_LENGTH = 26
HIDDEN = 512
INPUT = 1024
BATCH = 2048

NCORES = 8
NSHARD = 4                 # batch shards per direction group
BC = BATCH // NSHARD       # 512 batch rows per core

P = 128
KI = INPUT // P            # 8  k-tiles for the input projection
KH = HIDDEN // P           # 4  k-tiles for the recurrent matmul
NJ = HIDDEN // P           # 4  hidden chunks
NM = 4 * HIDDEN // P       # 16 gate m-tiles

# "f32r": fp32 storage, PE in float32r (full-rate at N>=256, ~tf32
#          precision).  227 ns/MM sustained (fp32-width LDWEIGHTS is not
#          fully hidden), rel err ~6e-4.
# "bf16": bf16 storage+PE (half DMA/SBUF), fp32 PSUM accumulation.
#          216 ns/MM sustained (FWL hides LDWEIGHTS), rel err ~9.6e-3.
# "mixed": x/w_ih bf16, h/w_hh f32r -- each bf16<->f32r group switch
#          exposes one LDWEIGHTS (~190ns), netting worse than pure bf16.
MM_MODE = os.environ.get("BASS_LSTM_MM", "bf16")

_CACHE = {}


def _build(T, Bc, mode):
    import concourse.mybir as mybir
    import concourse.tile as tile
    from concourse import bacc

    dt = mybir.dt
    AF = mybir.ActivationFunctionType

    # matmul-operand storage dtypes; the BIR verifier requires fp32r matmul
    # inputs to be produced as fp32r, so the recurrent path carries rec_dt
    # end-to-end.  x_dt covers the input-projection operands (x, w_ih).
    if mode == "bf16":
        x_dt = rec_dt = dt.bfloat16
    elif mode == "mixed":
        x_dt, rec_dt = dt.bfloat16, dt.float32r
    else:
        x_dt = rec_dt = dt.float32r

    nc = bacc.Bacc("TRN2", target_bir_lowering=False, debug=False,
                   num_devices=NCORES)
    xT = nc.dram_tensor("xT", [T, INPUT, Bc], x_dt, kind="ExternalInput").ap()
    w_ih = nc.dram_tensor("w_ih", [INPUT, 4 * HIDDEN], x_dt,
                          kind="ExternalInput").ap()
    w_hh = nc.dram_tensor("w_hh", [HIDDEN, 4 * HIDDEN], rec_dt,
                          kind="ExternalInput").ap()
    bias = nc.dram_tensor("bias", [P, NM], dt.float32, kind="ExternalInput").ap()
    out_dt = rec_dt
    yT = nc.dram_tensor("yT", [T, HIDDEN, Bc], out_dt, kind="ExternalOutput").ap()

    with tile.TileContext(nc) as tc:
        with tc.tile_pool(name="wpool", bufs=1) as wp, \
             tc.tile_pool(name="xpool", bufs=2) as xp, \
             tc.tile_pool(name="state", bufs=2) as sp, \
             tc.tile_pool(name="gates", bufs=2) as gp, \
             tc.tile_pool(name="tmp", bufs=2) as tp, \
             tc.tile_pool(name="psum", bufs=2, space="PSUM") as pp:

            # Startup-latency ordering: the t=0 x tile and the j=0 weight
            # chunk are DMA'd first so the PE can start ~35us earlier than
            # if all weights were queued at once (the SDMA rings drain all
            # queued transfers at a fair share, so everything would land
            # together). Host pre-permutes W columns j-major (see
            # _prep_inputs) so each j-chunk is one contiguous DMA.
            w_ih_r = w_ih.rearrange("(k p) n -> p k n", p=P)
            w_hh_r = w_hh.rearrange("(k p) n -> p k n", p=P)

            # PE_HAM warm-up: the PE clock gate defaults to K=4/8 (half
            # rate) and only releases after ~3.4us of continuous HIGH
            # activity, so the first ~20 real matmuls would run at 427ns
            # instead of 216ns.  Burn that ramp on dummy matmuls over
            # zeroed scratch tiles while the critical x/w DMAs are still
            # in flight.  The dummies must be full-width (N=512, ~100%
            # streaming occupancy): short N=64 dummies are dispatch-bound
            # at ~51% busy, which HAM reads as low activity and re-
            # throttles mid-warmup.  40 x 216ns ends ~15.5us in, right
            # when the 2MB critical DMA wave lands.
            # (memset can't produce float32r, so bf16-path only.)
            if x_dt == dt.bfloat16:
                warm_w = wp.tile([P, P], x_dt, tag="warm")
                nc.vector.memset(warm_w, 0.0)
                warm_x = wp.tile([P, Bc], x_dt, tag="warmx")
                nc.vector.memset(warm_x, 0.0)
                warm_ps = pp.tile([P, Bc], dt.float32, tag="ps0")
                for _ in range(32):
                    nc.tensor.matmul(warm_ps, lhsT=warm_w, rhs=warm_x,
                                     start=True, stop=True)

            # Startup ramp: t=0 x and w_ih arrive as per-k slices so the PE
            # can start on the first (x,w) pair as soon as it lands.  The
            # DMA rings serve queued tiles round-robin, so the 16 critical
            # tiles are split across the sync queue (weights) and the
            # scalar queue (x) to land in parallel, and everything
            # non-critical (bias, xt1, w_hh, w_ih j>0) is queued after.
            xt0k = []
            w_ih_sb = [[None] * KI for _ in range(NJ)]
            for k in range(KI):
                xk = wp.tile([P, Bc], x_dt, tag=f"x0_{k}")
                nc.scalar.dma_start(
                    out=xk, in_=xT[0, k * P:(k + 1) * P, :])
                xt0k.append(xk)
                wt = wp.tile([P, 4 * P], x_dt, tag=f"wih0_{k}")
                nc.sync.dma_start(out=wt, in_=w_ih_r[:, k, 0:4 * P])
                w_ih_sb[0][k] = wt

            bias_sb = wp.tile([P, NM], dt.float32, tag="bias")
            nc.sync.dma_start(out=bias_sb, in_=bias[:, :])

            # Remaining weights: w_hh rides the scalar queue behind the x0
            # slices, w_ih j>0 rides the sync queue behind the j=0 slices.
            # j>0 chunks are single [P, KI, 4P] tiles (not per-k like j=0):
            # they aren't startup-critical, and every SBUF tile costs a
            # semaphore whose teardown serializes in the ~10us epilogue
            # that counts toward measured exec time.
            w_hh_sb = []
            for j in range(NJ):
                wt = wp.tile([P, KH, 4 * P], rec_dt, tag=f"whh{j}")
                nc.scalar.dma_start(
                    out=wt, in_=w_hh_r[:, :, j * 4 * P:(j + 1) * 4 * P])
                w_hh_sb.append(wt)
            for j in range(1, NJ):
                for k in range(KI):
                    wt = wp.tile([P, 4 * P], x_dt, tag=f"wih{j}_{k}")
                    nc.sync.dma_start(
                        out=wt, in_=w_ih_r[:, k, j * 4 * P:(j + 1) * 4 * P])
                    w_ih_sb[j][k] = wt
            # t=1's x prefetch is not needed until ~44us in; queue it last
            # so it doesn't steal round-robin bandwidth from the weights.
            xt1 = None
            if T > 1:
                xt1 = xp.tile([P, KI, Bc], x_dt, tag="x")
                nc.sync.dma_start(
                    out=xt1, in_=xT[1].rearrange("(k p) b -> p k b", p=P))

            # h0 = c0 = 0, so step 0 skips the recurrent matmuls and the
            # f*c term entirely -- no initial state tiles needed (memset
            # can't produce float32r anyway).
            h_cur, c_cur = [], []

            GATE_FUNCS = (AF.Sigmoid, AF.Sigmoid, AF.Tanh, AF.Sigmoid)

            for t in range(T):
                if t == 1:
                    xt = xt1
                elif t > 1:
                    xt = xp.tile([P, KI, Bc], x_dt, tag="x")
                    nc.sync.dma_start(
                        out=xt, in_=xT[t].rearrange("(k p) b -> p k b", p=P))

                h_next, c_next = [], []
                for j in range(NJ):
                    acts = []
                    for gi in range(4):
                        m = gi * NJ + j
                        ps = pp.tile([P, Bc], dt.float32, tag=f"ps{gi}")

                        def mm_x(first):
                            for k in range(KI):
                                nc.tensor.matmul(
                                    ps,
                                    lhsT=w_ih_sb[j][k][:, gi * P:(gi + 1) * P],
                                    rhs=(xt0k[k] if t == 0 else xt[:, k, :]),
                                    start=(first and k == 0),
                                    stop=(not first and k == KI - 1))

                        def mm_h(first):
                            for k in range(KH):
                                nc.tensor.matmul(
                                    ps,
                                    lhsT=w_hh_sb[j][:, k, gi * P:(gi + 1) * P],
                                    rhs=h_cur[k],
                                    start=(first and k == 0),
                                    stop=(not first and k == KH - 1))

                        # Alternate x/h group order between consecutive gate
                        # tiles so same-dtype matmul groups chain across tile
                        # boundaries (bf16<->f32r mode switches expose the
                        # first LDWEIGHTS of the new group, ~420ns each).
                        if t == 0:
                            for k in range(KI):
                                nc.tensor.matmul(
                                    ps,
                                    lhsT=w_ih_sb[j][k][:, gi * P:(gi + 1) * P],
                                    rhs=xt0k[k],
                                    start=(k == 0), stop=(k == KI - 1))
                        elif gi % 2 == 0:
                            mm_x(True)
                            mm_h(False)
                        else:
                            mm_h(True)
                            mm_x(False)
                        gt = gp.tile([P, Bc], dt.float32, tag=f"g{gi}")
                        nc.scalar.activation(gt, ps, GATE_FUNCS[gi],
                                             bias=bias_sb[:, m:m + 1])
                        acts.append(gt)
                    i_t, f_t, g_t, o_t = acts
                    cn = sp.tile([P, Bc], dt.float32, tag=f"c{j}")
                    if t == 0:
                        nc.vector.tensor_mul(cn, i_t, g_t)
                    else:
                        u = tp.tile([P, Bc], dt.float32, tag="u")
                        nc.vector.tensor_mul(u, i_t, g_t)
                        v = tp.tile([P, Bc], dt.float32, tag="v")
                        nc.vector.tensor_mul(v, f_t, c_cur[j])
                        nc.vector.tensor_add(cn, u, v)
                    th = tp.tile([P, Bc], dt.float32, tag="th")
                    hn = sp.tile([P, Bc], rec_dt, tag=f"h{j}")
                    if t == T - 1:
                        # Last step: h feeds only the output DMA, so split
                        # the tanh/mul/store chain into half-width pieces --
                        # the first half's DMA starts while the second half
                        # is still in the ACT/DVE pipe, shortening the
                        # terminal drain after the final matmul.
                        H2 = Bc // 2
                        for s0 in (0, H2):
                            sl = slice(s0, s0 + H2)
                            nc.scalar.activation(th[:, sl], cn[:, sl],
                                                 AF.Tanh)
                            nc.vector.tensor_mul(hn[:, sl], o_t[:, sl],
                                                 th[:, sl])
                            nc.sync.dma_start(
                                out=yT[t, j * P:(j + 1) * P, sl],
                                in_=hn[:, sl])
                    else:
                        nc.scalar.activation(th, cn, AF.Tanh)
                        nc.vector.tensor_mul(hn, o_t, th)
                        nc.sync.dma_start(out=yT[t, j * P:(j + 1) * P, :],
                                          in_=hn)
                    h_next.append(hn)
                    c_next.append(cn)
                h_cur, c_cur = h_next, c_next

    nc.compile()
    return nc


def _get_nc(T=FRAME_LENGTH, Bc=BC, mode=MM_MODE):
    key = (T, Bc, mode)
    if key not in _CACHE:
        _CACHE[key] = _build(T, Bc, mode)
    return _CACHE[key]


def _prep_inputs(embed_feats, w_ih_l, w_hh_l, b_ih_l, b_hh_l,
                 w_ih_r, w_hh_r, b_ih_r, b_hh_r, mode):
    import ml_dtypes

    if mode == "bf16":
        x_np = rec_np = ml_dtypes.bfloat16
    elif mode == "mixed":
        x_np, rec_np = ml_dtypes.bfloat16, np.float32
    else:
        x_np = rec_np = np.float32
    T = embed_feats.shape[1]

    w = {
        0: (np.asarray(w_ih_l), np.asarray(w_hh_l),
            np.asarray(b_ih_l) + np.asarray(b_hh_l)),
        1: (np.asarray(w_ih_r), np.asarray(w_hh_r),
            np.asarray(b_ih_r) + np.asarray(b_hh_r)),
    }
    x = np.asarray(embed_feats)

    # j-major column permutation of the 4H gate dim: block j holds the four
    # gates' columns for hidden chunk j, so each j-chunk loads contiguously
    j_idx, g_idx, c_idx = np.meshgrid(
        np.arange(NJ), np.arange(4), np.arange(P), indexing="ij")
    perm = (g_idx * (NJ * P) + j_idx * P + c_idx).reshape(-1)

    in_maps = []
    for c in range(NCORES):
        d, s = c // NSHARD, c % NSHARD
        xs = x[s * BC:(s + 1) * BC]
        if d == 1:
            xs = xs[:, ::-1]
        xT = np.ascontiguousarray(xs.transpose(1, 2, 0)).astype(x_np)
        w_ihT = np.ascontiguousarray(w[d][0].T[:, perm]).astype(x_np)
        w_hhT = np.ascontiguousarray(w[d][1].T[:, perm]).astype(rec_np)
        bias = np.ascontiguousarray(
            w[d][2].astype(np.float32).reshape(NM, P).T)
        in_maps.append({"xT": xT, "w_ih": w_ihT, "w_hh": w_hhT, "bias": bias})
    return in_maps, T


def _run(inputs, mode=MM_MODE, trace=False, trace_kwargs=None):
    from concourse.bass_utils import run_bass_kernel_spmd

    in_maps, T = _prep_inputs(mode=mode, **inputs)
    nc = _get_nc(T=T, mode=mode)
    res = run_bass_kernel_spmd(nc, in_maps, list(range(NCORES)),
                               trace=trace, **(trace_kwargs or {}))

    out = np.empty((BATCH, T, 2 * HIDDEN), np.float32)
    for c in range(NCORES):
        d, s = c // NSHARD, c % NSHARD
        yt = np.asarray(res.results[c]["yT"], dtype=np.float32)  # [T, H, Bc]
        arr = yt.transpose(2, 0, 1)                              # [Bc, T, H]
        if d == 1:
            arr = arr[:, ::-1]
        out[s * BC:(s + 1) * BC, :, d * HIDDEN:(d + 1) * HIDDEN] = arr
    return out, res


def kernel(**inputs):
    out, _ = _run(inputs)
    return out



# revision 17
# speedup vs baseline: 1.0014x; 1.0014x over previous
"""Bidirectional LSTM encoder (nn_BiEncode) as a Bass/Tile kernel on 8 trn2 cores.

Sharding: direction-split x batch-split. Cores 0-3 run the LEFT (forward-time)
direction on batch shards 0-3 (512 rows each); cores 4-7 run the RIGHT
direction (time-reversed input, handled host-side) on the same batch shards.
Every core runs the identical SPMD program; direction differences live
entirely in the data it is fed (weights + time-reversed x).

Device layout: everything is kept "transposed" (feature dim on partitions,
batch on the free dim) so the scan needs no on-chip transposes:
  x fed as xT[t, i, b], weights as W^T, h/c as [H, B] tiles, output written
  as yT[t, h, b] and un-transposed on the host.

Per timestep the full gate pre-activation g^T[4H, B] is computed as 12
PSUM-accumulated matmuls per 128-row gate tile (8 k-tiles of x-projection +
4 k-tiles of the recurrent term) -- the input projection is fused into the
scan, so no pre-activation tensor is ever materialized. ACT applies
sigmoid/tanh straight out of PSUM; DVE does the cell update.
"""

import os

import numpy as np

FRAME_LENGTH = 26
HIDDEN = 512
INPUT = 1024
BATCH = 2048

NCORES = 8
NSHARD = 4                 # batch shards per direction group
BC = BATCH // NSHARD       # 512 batch rows per core

P = 128
KI = INPUT // P            # 8  k-tiles for the input projection
KH = HIDDEN // P           # 4  k-tiles for the recurrent matmul
NJ = HIDDEN // P           # 4  hidden chunks
NM = 4 * HIDDEN // P       # 16 gate m-tiles

# "f32r": fp32 storage, PE in float32r (full-rate at N>=256, ~tf32
#          precision).  227 ns/MM sustained (fp32-width LDWEIGHTS is not
#          fully hidden), rel err ~6e-4.
# "bf16": bf16 storage+PE (half DMA/SBUF), fp32 PSUM accumulation.
#          216 ns/MM sustained (FWL hides LDWEIGHTS), rel err ~9.6e-3.
# "mixed": x/w_ih bf16, h/w_hh f32r -- each bf16<->f32r group switch
#          exposes one LDWEIGHTS (~190ns), netting worse than pure bf16.
MM_MODE = os.environ.get("BASS_LSTM_MM", "bf16")

_CACHE = {}


def _build(T, Bc, mode):
    import concourse.mybir as mybir
    import concourse.tile as tile
    from concourse import bacc

    dt = mybir.dt
    AF = mybir.ActivationFunctionType

    # matmul-operand storage dtypes; the BIR verifier requires fp32r matmul
    # inputs to be produced as fp32r, so the recurrent path carries rec_dt
    # end-to-end.  x_dt covers the input-projection operands (x, w_ih).
    if mode == "bf16":
        x_dt = rec_dt = dt.bfloat16
    elif mode == "mixed":
        x_dt, rec_dt = dt.bfloat16, dt.float32r
    else:
        x_dt = rec_dt = dt.float32r

    nc = bacc.Bacc("TRN2", target_bir_lowering=False, debug=False,
                   num_devices=NCORES)
    xT = nc.dram_tensor("xT", [T, INPUT, Bc], x_dt, kind="ExternalInput").ap()
    w_ih = nc.dram_tensor("w_ih", [INPUT, 4 * HIDDEN], x_dt,
                          kind="ExternalInput").ap()
    w_hh = nc.dram_tensor("w_hh", [HIDDEN, 4 * HIDDEN], rec_dt,
                          kind="ExternalInput").ap()
    bias = nc.dram_tensor("bias", [P, NM], dt.float32, kind="ExternalInput").ap()
    out_dt = rec_dt
    yT = nc.dram_tensor("yT", [T, HIDDEN, Bc], out_dt, kind="ExternalOutput").ap()

    with tile.TileContext(nc) as tc:
        with tc.tile_pool(name="wpool", bufs=1) as wp, \
             tc.tile_pool(name="xpool", bufs=2) as xp, \
             tc.tile_pool(name="state", bufs=2) as sp, \
             tc.tile_pool(name="gates", bufs=2) as gp, \
             tc.tile_pool(name="tmp", bufs=2) as tp, \
             tc.tile_pool(name="psum", bufs=2, space="PSUM") as pp:

            # Startup-latency ordering: the t=0 x tile and the j=0 weight
            # chunk are DMA'd first so the PE can start ~35us earlier than
            # if all weights were queued at once (the SDMA rings drain all
            # queued transfers at a fair share, so everything would land
            # together). Host pre-permutes W columns j-major (see
            # _prep_inputs) so each j-chunk is one contiguous DMA.
            w_ih_r = w_ih.rearrange("(k p) n -> p k n", p=P)
            w_hh_r = w_hh.rearrange("(k p) n -> p k n", p=P)

            # PE_HAM warm-up: the PE clock gate defaults to K=4/8 (half
            # rate) and only releases after ~3.4us of continuous HIGH
            # activity, so the first ~20 real matmuls would run at 427ns
            # instead of 216ns.  Burn that ramp on dummy matmuls over
            # zeroed scratch tiles while the critical x/w DMAs are still
            # in flight.  The dummies must be full-width (N=512, ~100%
            # streaming occupancy): short N=64 dummies are dispatch-bound
            # at ~51% busy, which HAM reads as low activity and re-
            # throttles mid-warmup.  40 x 216ns ends ~15.5us in, right
            # when the 2MB critical DMA wave lands.
            # (memset can't produce float32r, so bf16-path only.)
            if x_dt == dt.bfloat16:
                warm_w = wp.tile([P, P], x_dt, tag="warm")
                nc.vector.memset(warm_w, 0.0)
                warm_x = wp.tile([P, Bc], x_dt, tag="warmx")
                nc.vector.memset(warm_x, 0.0)
                warm_ps = pp.tile([P, Bc], dt.float32, tag="ps0")
                for _ in range(40):
                    nc.tensor.matmul(warm_ps, lhsT=warm_w, rhs=warm_x,
                                     start=True, stop=True)

            # Startup ramp: t=0 x and w_ih arrive as per-k slices so the PE
            # can start on the first (x,w) pair as soon as it lands.  The
            # DMA rings serve queued tiles round-robin, so the 16 critical
            # tiles are split across the sync queue (weights) and the
            # scalar queue (x) to land in parallel, and everything
            # non-critical (bias, xt1, w_hh, w_ih j>0) is queued after.
            xt0k = []
            w_ih_sb = [[None] * KI for _ in range(NJ)]
            for k in range(KI):
                xk = wp.tile([P, Bc], x_dt, tag=f"x0_{k}")
                nc.scalar.dma_start(
                    out=xk, in_=xT[0, k * P:(k + 1) * P, :])
                xt0k.append(xk)
                wt = wp.tile([P, 4 * P], x_dt, tag=f"wih0_{k}")
                nc.sync.dma_start(out=wt, in_=w_ih_r[:, k, 0:4 * P])
                w_ih_sb[0][k] = wt

            bias_sb = wp.tile([P, NM], dt.float32, tag="bias")
            nc.sync.dma_start(out=bias_sb, in_=bias[:, :])

            # Remaining weights: w_hh rides the scalar queue behind the x0
            # slices, w_ih j>0 rides the sync queue behind the j=0 slices.
            # j>0 chunks are single [P, KI, 4P] tiles (not per-k like j=0):
            # they aren't startup-critical, and every SBUF tile costs a
            # semaphore whose teardown serializes in the ~10us epilogue
            # that counts toward measured exec time.
            w_hh_sb = []
            for j in range(NJ):
                wt = wp.tile([P, KH, 4 * P], rec_dt, tag=f"whh{j}")
                nc.scalar.dma_start(
                    out=wt, in_=w_hh_r[:, :, j * 4 * P:(j + 1) * 4 * P])
                w_hh_sb.append(wt)
            for j in range(1, NJ):
                for k in range(KI):
                    wt = wp.tile([P, 4 * P], x_dt, tag=f"wih{j}_{k}")
                    nc.sync.dma_start(
                        out=wt, in_=w_ih_r[:, k, j * 4 * P:(j + 1) * 4 * P])
                    w_ih_sb[j][k] = wt
            # t=1's x prefetch is not needed until ~44us in; queue it last
            # so it doesn't steal round-robin bandwidth from the weights.
            xt1 = None
            if T > 1:
                xt1 = xp.tile([P, KI, Bc], x_dt, tag="x")
                nc.sync.dma_start(
                    out=xt1, in_=xT[1].rearrange("(k p) b -> p k b", p=P))

            # h0 = c0 = 0, so step 0 skips the recurrent matmuls and the
            # f*c term entirely -- no initial state tiles needed (memset
            # can't produce float32r anyway).
            h_cur, c_cur = [], []

            GATE_FUNCS = (AF.Sigmoid, AF.Sigmoid, AF.Tanh, AF.Sigmoid)

            for t in range(T):
                if t == 1:
                    xt = xt1
                elif t > 1:
                    xt = xp.tile([P, KI, Bc], x_dt, tag="x")
                    nc.sync.dma_start(
                        out=xt, in_=xT[t].rearrange("(k p) b -> p k b", p=P))

                h_next, c_next = [], []
                for j in range(NJ):
                    acts = []
                    for gi in range(4):
                        m = gi * NJ + j
                        ps = pp.tile([P, Bc], dt.float32, tag=f"ps{gi}")

                        def mm_x(first):
                            for k in range(KI):
                                nc.tensor.matmul(
                                    ps,
                                    lhsT=w_ih_sb[j][k][:, gi * P:(gi + 1) * P],
                                    rhs=(xt0k[k] if t == 0 else xt[:, k, :]),
                                    start=(first and k == 0),
                                    stop=(not first and k == KI - 1))

                        def mm_h(first):
                            for k in range(KH):
                                nc.tensor.matmul(
                                    ps,
                                    lhsT=w_hh_sb[j][:, k, gi * P:(gi + 1) * P],
                                    rhs=h_cur[k],
                                    start=(first and k == 0),
                                    stop=(not first and k == KH - 1))

                        # Alternate x/h group order between consecutive gate
                        # tiles so same-dtype matmul groups chain across tile
                        # boundaries (bf16<->f32r mode switches expose the
                        # first LDWEIGHTS of the new group, ~420ns each).
                        if t == 0:
                            for k in range(KI):
                                nc.tensor.matmul(
                                    ps,
                                    lhsT=w_ih_sb[j][k][:, gi * P:(gi + 1) * P],
                                    rhs=xt0k[k],
                                    start=(k == 0), stop=(k == KI - 1))
                        elif gi % 2 == 0:
                            mm_x(True)
                            mm_h(False)
                        else:
                            mm_h(True)
                            mm_x(False)
                        gt = gp.tile([P, Bc], dt.float32, tag=f"g{gi}")
                        nc.scalar.activation(gt, ps, GATE_FUNCS[gi],
                                             bias=bias_sb[:, m:m + 1])
                        acts.append(gt)
                    i_t, f_t, g_t, o_t = acts
                    cn = sp.tile([P, Bc], dt.float32, tag=f"c{j}")
                    if t == 0:
                        nc.vector.tensor_mul(cn, i_t, g_t)
                    else:
                        u = tp.tile([P, Bc], dt.float32, tag="u")
                        nc.vector.tensor_mul(u, i_t, g_t)
                        v = tp.tile([P, Bc], dt.float32, tag="v")
                        nc.vector.tensor_mul(v, f_t, c_cur[j])
                        nc.vector.tensor_add(cn, u, v)
                    th = tp.tile([P, Bc], dt.float32, tag="th")
                    hn = sp.tile([P, Bc], rec_dt, tag=f"h{j}")
                    if t == T - 1:
                        # Last step: h feeds only the output DMA, so split
                        # the tanh/mul/store chain into half-width pieces --
                        # the first half's DMA starts while the second half
                        # is still in the ACT/DVE pipe, shortening the
                        # terminal drain after the final matmul.
                        H2 = Bc // 2
                        for s0 in (0, H2):
                            sl = slice(s0, s0 + H2)
                            nc.scalar.activation(th[:, sl], cn[:, sl],
                                                 AF.Tanh)
                            nc.vector.tensor_mul(hn[:, sl], o_t[:, sl],
                                                 th[:, sl])
                            nc.sync.dma_start(
                                out=yT[t, j * P:(j + 1) * P, sl],
                                in_=hn[:, sl])
                    else:
                        nc.scalar.activation(th, cn, AF.Tanh)
                        nc.vector.tensor_mul(hn, o_t, th)
                        nc.sync.dma_start(out=yT[t, j * P:(j + 1) * P, :],
                                          in_=hn)
                    h_next.append(hn)
                    c_next.append(cn)
                h_cur, c_cur = h_next, c_next

    nc.compile()
    return nc


def _get_nc(T=FRAME_LENGTH, Bc=BC, mode=MM_MODE):
    key = (T, Bc, mode)
    if key not in _CACHE:
        _CACHE[key] = _build(T, Bc, mode)
    return _CACHE[key]


def _prep_inputs(embed_feats, w_ih_l, w_hh_l, b_ih_l, b_hh_l,
                 w_ih_r, w_hh_r, b_ih_r, b_hh_r, mode):
    import ml_dtypes

    if mode == "bf16":
        x_np = rec_np = ml_dtypes.bfloat16
    elif mode == "mixed":
        x_np, rec_np = ml_dtypes.bfloat16, np.float32
    else:
        x_np = rec_np = np.float32
    T = embed_feats.shape[1]

    w = {
        0: (np.asarray(w_ih_l), np.asarray(w_hh_l),
            np.asarray(b_ih_l) + np.asarray(b_hh_l)),
        1: (np.asarray(w_ih_r), np.asarray(w_hh_r),
            np.asarray(b_ih_r) + np.asarray(b_hh_r)),
    }
    x = np.asarray(embed_feats)

    # j-major column permutation of the 4H gate dim: block j holds the four
    # gates' columns for hidden chunk j, so each j-chunk loads contiguously
    j_idx, g_idx, c_idx = np.meshgrid(
        np.arange(NJ), np.arange(4), np.arange(P), indexing="ij")
    perm = (g_idx * (NJ * P) + j_idx * P + c_idx).reshape(-1)

    in_maps = []
    for c in range(NCORES):
        d, s = c // NSHARD, c % NSHARD
        xs = x[s * BC:(s + 1) * BC]
        if d == 1:
            xs = xs[:, ::-1]
        xT = np.ascontiguousarray(xs.transpose(1, 2, 0)).astype(x_np)
        w_ihT = np.ascontiguousarray(w[d][0].T[:, perm]).astype(x_np)
        w_hhT = np.ascontiguousarray(w[d][1].T[:, perm]).astype(rec_np)
        bias = np.ascontiguousarray(
            w[d][2].astype(np.float32).reshape(NM, P).T)
        in_maps.append({"xT": xT, "w_ih": w_ihT, "w_hh": w_hhT, "bias": bias})
    return in_maps, T


def _run(inputs, mode=MM_MODE, trace=False, trace_kwargs=None):
    from concourse.bass_utils import run_bass_kernel_spmd

    in_maps, T = _prep_inputs(mode=mode, **inputs)
    nc = _get_nc(T=T, mode=mode)
    res = run_bass_kernel_spmd(nc, in_maps, list(range(NCORES)),
                               trace=trace, **(trace_kwargs or {}))

    out = np.empty((BATCH, T, 2 * HIDDEN), np.float32)
    for c in range(NCORES):
        d, s = c // NSHARD, c % NSHARD
        yt = np.asarray(res.results[c]["yT"], dtype=np.float32)  # [T, H, Bc]
        arr = yt.transpose(2, 0, 1)                              # [Bc, T, H]
        if d == 1:
            arr = arr[:, ::-1]
        out[s * BC:(s + 1) * BC, :, d * HIDDEN:(d + 1) * HIDDEN] = arr
    return out, res


def kernel(**inputs):
    out, _ = _run(inputs)
    return out

